# revision 30
# baseline (speedup 1.0000x reference)
"""Trainium2 Bass kernel for the EighMSE loss (data-parallel over 8 cores).

Math (row (a, b, c) encodes [[a, b], [b, c]]):
  SM = a + c, DF = a - c, RT = sqrt(DF^2 + 4 b^2)
  closed-form evals = (SM +- RT) / 2,  x = clamp(DF / RT, -1, 1)
  n0 = sqrt((1 + x)/2), n1u = sqrt((1 - x)/2)
  LAPACK larger-eval eigenvector g = tau0 * (n0, s_b * n1u) with
    tau0 = -1 if DF > 0 else s_b * s_SM,  s_* = sign
  smaller-eval eigenvector = s_SM * (-g1, g0)

Sign trick: tau0 = -sign(Z) with Z = relu(DF)*2^40 - b*SM, so the pair
products need only sign(Zp*Zt) (and * sign(bp*bt) for the g1 column) —
no per-half mask chains.  Sum identities (per-core partials, f32 accums):
  RT2S = sum RT^2 (both halves)     WWS = sum sqrt(RT2p*RT2t)
  U = sum (1+xp)(1+xt)              V = sum (1-xp)(1-xt)
  SP0 = sum sgn(ZZ) * NN0           SP1 = sum sgn(ZZ*BB) * NN1
  SP0m = sum [SMp*SMt<0] * P0       SP1m = sum [SMp*SMt<0] * P1
  A = sum dSM^2   C = sum dDF^2   D = sum db^2
with NN0 = n0p*n0t = sqrt(U-term)/2, NN1 = sqrt(V-term)/2.
Host combine:
  Sx = (U - V)/2;  E1 = B + Sx/2 - 2 SP0;  E2 = B - Sx/2 - 2 SP1
  F0 = E1 + 4 SP0m;  F1 = E2 + 4 SP1m;  Bs = RT2S - 2 WWS
  loss = w0 (A+Bs)/(4B) + w1 E1/B + w2 E2/B + w3 F1/B + w4 F0/B
         + w5 (A/2 + C/2 + D)/(3B)

Engine split: Pool does SM/DF/b*SM, Act does the bf16 convert + sqrts,
DVE runs fused custom ops (square-diff-accumulate, signed-mul-accumulate,
masked-accumulate, clamped divide via 1-Newton reciprocal).
"""

import numpy as np
from contextlib import ExitStack
from operator import add as _opadd

import concourse.bass as bass
import concourse.bacc as bacc
import concourse.tile as tile
from concourse import mybir
from concourse import dve_ops as _D
from concourse.bass_utils import run_bass_kernel_spmd
from concourse.dve_spec import (
    AluOp,
    Bin,
    C0,
    C1,
    C2,
    One,
    Spec,
    Src0,
    Src1,
    Zero,
    _has_src1,
    lower,
    maxx,
    minn,
    select,
    sq,
)
from concourse.dve_uop import DveOpSpec

F32 = mybir.dt.float32
BF16 = mybir.dt.bfloat16
OP = mybir.AluOpType
AF = mybir.ActivationFunctionType

B_TOTAL = 4_194_304
NCORES = 8
S = B_TOTAL // NCORES          # samples per core
P = 128                        # partitions
NPC = S // P                   # samples per partition (4096)
W = 1024                       # sample-pairs per tile per partition
CW = 2 * W                     # combined (pred|true) tile width
NT = NPC // W                  # tiles per core
NSTAT = 7                      # chained f32 accumulator columns
BIGS = float(2.0 ** 40)        # relu(DF) scale for the sign trick

# ---- custom DVE op registration (process-local registries) -----------------

_VER = "v3"


def _acc_ref(body_fn):
    def _r(in0, in1, c0, c1, c2):
        b = body_fn(in0, in1, c0, c1, c2).astype(np.float32)
        return b, c0 + b.reshape(b.shape[0], -1).sum(axis=-1, keepdims=True)

    return _r


def _register(name, spec):
    if name in _D._SUB_OPCODE_FOR_NAME:
        return next(op for op in _D.OPS if op.name == name)
    row = _D._CUSTOM_DVE_ROW_BASE + len(_D.OPS)
    assert row <= 0x1F, f"custom-DVE row overflow: {row}"
    _D._SUB_OPCODE_FOR_NAME[name] = row
    uops = lower(spec, ver=_VER)
    sha = DveOpSpec(name=name, opcode=row, uops=uops, rd1_en=_has_src1(spec)).sha(_VER)
    op = _D.DveOp(name, spec, subdim=False, uops_sha={_VER: sha})
    _D.OPS.append(op)
    _D.CUSTOM_DVE_SPECS[name] = spec
    return op


_nx = Bin(AluOp.BITWISE_NOT, Src1, Src1)
_y0 = _nx * C0
_y1 = _y0 * (C1 - Src1 * _y0)


def _ref_xop(in0, in1, c0, c1, c2):
    nx = (~np.ascontiguousarray(in1, np.float32).view(np.int32)).view(np.float32)
    y0 = nx * np.float32(c0)
    y1 = y0 * (np.float32(c1) - in1 * y0)
    x = in0.astype(np.float32) * y1
    return np.minimum(np.maximum(x, c2), 1.0)


# out = clamp(in0 * recip1nr(in1), [imm2, 1]); in1 must be f32 (bit trick)
X_CLAMP_DIV = _register(
    "X_CLAMP_DIV", Spec(body=minn(maxx(Src0 * _y1, C2), One), reference=_ref_xop)
)
X_CONSTS = dict(s0=-0.23549792, s1=2.0017324, imm2=-1.0)

# out = in0^2 + in1^2 * s1; accum_out = s0 + sum(out)
RT2_ACC = _register(
    "RT2_ACC",
    Spec(
        body=sq(Src0) + sq(Src1) * C1,
        accum=_opadd,
        accum_init=C0,
        reference=_acc_ref(
            lambda in0, in1, c0, c1, c2: in0.astype(np.float32) ** 2
            + in1.astype(np.float32) ** 2 * c1
        ),
    ),
)

# out = (1+in0)(1+in1); accum_out = s0 + sum(out)
PAIR_U = _register(
    "PAIR_U",
    Spec(
        body=(Src0 + One) * (Src1 + One),
        accum=_opadd,
        accum_init=C0,
        reference=_acc_ref(
            lambda in0, in1, c0, c1, c2: (in0.astype(np.float32) + 1.0) * (in1 + 1.0)
        ),
    ),
)

# out = (1-in0)(1-in1); accum_out = s0 + sum(out)
PAIR_V = _register(
    "PAIR_V",
    Spec(
        body=(One - Src0) * (One - Src1),
        accum=_opadd,
        accum_init=C0,
        reference=_acc_ref(
            lambda in0, in1, c0, c1, c2: (1.0 - in0.astype(np.float32)) * (1.0 - in1)
        ),
    ),
)

# out = in0<0 ? -in1 : in1; accum_out = s0 + sum(out)
SGN_MUL_ACC = _register(
    "SGN_MUL_ACC",
    Spec(
        body=select(Src0 < Zero, Zero - Src1, Src1),
        accum=_opadd,
        accum_init=C0,
        reference=_acc_ref(
            lambda in0, in1, c0, c1, c2: np.where(
                in0 < 0, -in1.astype(np.float32), in1.astype(np.float32)
            )
        ),
    ),
)

# out = in0<0 ? in1 : 0; accum_out = s0 + sum(out)
MASK_ACC = _register(
    "MASK_ACC",
    Spec(
        body=select(Src0 < Zero, Src1, Zero),
        accum=_opadd,
        accum_init=C0,
        reference=_acc_ref(
            lambda in0, in1, c0, c1, c2: np.where(in0 < 0, in1.astype(np.float32), 0.0)
        ),
    ),
)

# out = (in0-in1)^2; accum_out = s0 + sum(out)
SQDIFF_ACC = _register(
    "SQDIFF_ACC",
    Spec(
        body=sq(Src0 - Src1),
        accum=_opadd,
        accum_init=C0,
        reference=_acc_ref(
            lambda in0, in1, c0, c1, c2: (in0.astype(np.float32) - in1) ** 2
        ),
    ),
)

# stats column indices
RT2S, UCOL, VCOL, SP0, SP1, SP0M, SP1M = range(NSTAT)

_BUILT = None


def _build_bass():
    nc = bacc.Bacc()
    yp = nc.declare_dram_parameter("y_pred", [S, 3], F32, isOutput=False)
    yt = nc.declare_dram_parameter("y_true", [S, 3], F32, isOutput=False)
    out = nc.declare_dram_parameter("out", [P, NSTAT + 4], F32, isOutput=True)

    ypr = yp.rearrange("(p n) c -> p n c", p=P)
    ytr = yt.rearrange("(p n) c -> p n c", p=P)

    with tile.TileContext(nc) as tc, ExitStack() as ctx:
        inp = ctx.enter_context(tc.tile_pool(name="inp", bufs=2))
        wk = ctx.enter_context(tc.tile_pool(name="wk", bufs=2))
        accp = ctx.enter_context(tc.tile_pool(name="accp", bufs=1))

        stats = accp.tile([P, NSTAT * (NT + 1)], F32)
        nc.vector.memset(stats[:], 0.0)
        wwacc = accp.tile([P, NT], F32)
        dacc = accp.tile([P, 3 * NT], F32)
        epsc = accp.tile([P, 1], F32)
        nc.vector.memset(epsc[:], 1e-12)

        def st(col, i):
            k = col * (NT + 1) + i
            return stats[:, k : k + 1]

        for i in range(NT):
            xall = inp.tile([P, CW, 3], F32, tag="xall")
            nc.sync.dma_start(xall[:, 0:W, :], ypr[:, bass.ts(i, W), :])
            nc.sync.dma_start(xall[:, W:CW, :], ytr[:, bass.ts(i, W), :])
            a = xall[:, :, 0]
            b = xall[:, :, 1]
            c = xall[:, :, 2]

            SM = wk.tile([P, CW], BF16, tag="SM")
            DF = wk.tile([P, CW], BF16, tag="DF")
            bb = wk.tile([P, CW], BF16, tag="bb")
            RT2 = wk.tile([P, CW], BF16, tag="RT2")
            RT = wk.tile([P, CW], F32, tag="RT")
            BS = wk.tile([P, CW], BF16, tag="BS")
            Z = wk.tile([P, CW], BF16, tag="Z")
            XC = wk.tile([P, CW], BF16, tag="XC")

            def emit_chain(sl, colidx, dve_sub):
                aa, bsl, cc = xall[:, sl, 0], xall[:, sl, 1], xall[:, sl, 2]
                nc.gpsimd.tensor_add(SM[:, sl], aa, cc)
                if dve_sub:
                    nc.vector.tensor_sub(DF[:, sl], aa, cc)
                else:
                    nc.gpsimd.tensor_sub(DF[:, sl], aa, cc)
                nc.scalar.activation(bb[:, sl], bsl, AF.Copy)
                nc.vector._custom_dve(
                    RT2_ACC, out=RT2[:, sl], in0=DF[:, sl], in1=bb[:, sl],
                    s0=0.0, s1=4.0, accum_out=st(RT2S, colidx),
                )
                nc.scalar.activation(RT[:, sl], RT2[:, sl], AF.Sqrt, bias=epsc[:])
                nc.vector.tensor_mul(BS[:, sl], bb[:, sl], SM[:, sl])
                nc.vector.tensor_scalar(
                    Z[:, sl], DF[:, sl], 0.0, BIGS, op0=OP.max, op1=OP.mult
                )
                nc.vector.tensor_sub(Z[:, sl], Z[:, sl], BS[:, sl])
                nc.vector._custom_dve(
                    X_CLAMP_DIV, out=XC[:, sl], in0=DF[:, sl], in1=RT[:, sl],
                    **X_CONSTS
                )

            if i == 0:
                emit_chain(slice(0, W), 0, True)
                emit_chain(slice(W, CW), NT, True)
            else:
                emit_chain(slice(0, CW), i, False)

            # tail: pred half [:, :W], true half [:, W:]
            def ph(t):
                return t[:, 0:W]

            def th(t):
                return t[:, W:CW]

            SS = wk.tile([P, W], BF16, tag="SS")
            nc.gpsimd.tensor_mul(SS[:], ph(SM), th(SM))
            BBp = wk.tile([P, W], BF16, tag="BBp")
            nc.gpsimd.tensor_mul(BBp[:], ph(bb), th(bb))
            da = wk.tile([P, W], BF16, tag="da")
            nc.vector.tensor_sub(da[:], ph(SM), th(SM))
            sa = wk.tile([P, W], BF16, tag="sa")
            nc.scalar.activation(sa[:], da[:], AF.Square, accum_out=dacc[:, 3 * i : 3 * i + 1])
            dc = wk.tile([P, W], BF16, tag="dc")
            nc.vector.tensor_sub(dc[:], ph(DF), th(DF))
            sc = wk.tile([P, W], BF16, tag="sc")
            nc.scalar.activation(sc[:], dc[:], AF.Square, accum_out=dacc[:, 3 * i + 1 : 3 * i + 2])
            dd = wk.tile([P, W], BF16, tag="dd")
            nc.vector.tensor_sub(dd[:], ph(bb), th(bb))
            sd = wk.tile([P, W], BF16, tag="sd")
            nc.scalar.activation(sd[:], dd[:], AF.Square, accum_out=dacc[:, 3 * i + 2 : 3 * i + 3])
            ww = wk.tile([P, W], BF16, tag="ww")
            nc.gpsimd.tensor_mul(ww[:], ph(RT2), th(RT2))
            wws = wk.tile([P, W], BF16, tag="wws")
            nc.scalar.activation(
                wws[:], ww[:], AF.Sqrt, accum_out=wwacc[:, i : i + 1]
            )
            u = wk.tile([P, W], BF16, tag="u")
            nc.vector._custom_dve(
                PAIR_U, out=u[:], in0=ph(XC), in1=th(XC),
                s0=0.0, accum_out=st(UCOL, i),
            )
            v = wk.tile([P, W], BF16, tag="v")
            nc.vector._custom_dve(
                PAIR_V, out=v[:], in0=ph(XC), in1=th(XC),
                s0=0.0, accum_out=st(VCOL, i),
            )
            NN0 = wk.tile([P, W], BF16, tag="NN0")
            nc.scalar.activation(NN0[:], u[:], AF.Sqrt, scale=0.25)
            NN1 = wk.tile([P, W], BF16, tag="NN1")
            nc.scalar.activation(NN1[:], v[:], AF.Sqrt, scale=0.25)

            ZZ = wk.tile([P, W], BF16, tag="ZZ")
            nc.vector.tensor_mul(ZZ[:], ph(Z), th(Z))
            P0 = wk.tile([P, W], BF16, tag="P0")
            nc.vector._custom_dve(
                SGN_MUL_ACC, out=P0[:], in0=ZZ[:], in1=NN0[:],
                s0=0.0, accum_out=st(SP0, i),
            )
            scr = wk.tile([P, W], BF16, tag="scr")
            nc.vector._custom_dve(
                MASK_ACC, out=scr[:], in0=SS[:], in1=P0[:],
                s0=0.0, accum_out=st(SP0M, i),
            )
            nc.vector.tensor_mul(BBp[:], ZZ[:], BBp[:])  # ZB in place
            P1 = wk.tile([P, W], BF16, tag="P1")
            nc.vector._custom_dve(
                SGN_MUL_ACC, out=P1[:], in0=BBp[:], in1=NN1[:],
                s0=0.0, accum_out=st(SP1, i),
            )
            scr2 = wk.tile([P, W], BF16, tag="scr2")
            nc.vector._custom_dve(
                MASK_ACC, out=scr2[:], in0=SS[:], in1=P1[:],
                s0=0.0, accum_out=st(SP1M, i),
            )



        outsums = accp.tile([P, NSTAT + 4], F32)
        stats3 = stats[:].rearrange("p (c t) -> p c t", c=NSTAT)
        rscr = accp.tile([P, NT + 1], F32)
        for cidx in range(NSTAT):
            nc.scalar.activation(
                rscr[:], stats3[:, cidx, :], AF.Copy,
                accum_out=outsums[:, cidx : cidx + 1],
            )
        rscr2 = accp.tile([P, NT], F32)
        nc.scalar.activation(
            rscr2[:], wwacc[:], AF.Copy, accum_out=outsums[:, NSTAT : NSTAT + 1]
        )
        dacc3 = dacc[:].rearrange("p (t k) -> p k t", k=3)
        rscr3 = accp.tile([P, NT], F32)
        for k in range(3):
            nc.scalar.activation(
                rscr3[:], dacc3[:, k, :], AF.Copy,
                accum_out=outsums[:, NSTAT + 1 + k : NSTAT + 2 + k],
            )
        nc.sync.dma_start(out[:, :], outsums[:])

    nc.compile()
    return nc


def _get_built():
    global _BUILT
    if _BUILT is None:
        _BUILT = _build_bass()
    return _BUILT


def _host_combine(nc, y_pred, y_true, weights):
    y_pred = np.ascontiguousarray(y_pred, dtype=np.float32)
    y_true = np.ascontiguousarray(y_true, dtype=np.float32)
    w = np.asarray(weights, dtype=np.float64)

    in_maps = []
    for cid in range(NCORES):
        in_maps.append(
            {
                "y_pred": y_pred[cid * S : (cid + 1) * S],
                "y_true": y_true[cid * S : (cid + 1) * S],
            }
        )
    res = run_bass_kernel_spmd(nc, in_maps, list(range(NCORES)))
    sums = np.zeros(NSTAT + 4, dtype=np.float64)
    for cid in range(NCORES):
        sums += np.asarray(res.results[cid]["out"], dtype=np.float64).sum(axis=0)

    rt2s, u, v, sp0, sp1, sp0m, sp1m, wws, A, C, D = sums
    Bn = float(B_TOTAL)
    sx = (u - v) / 2.0
    e1 = Bn + sx / 2.0 - 2.0 * sp0
    e2 = Bn - sx / 2.0 - 2.0 * sp1
    f0 = e1 + 4.0 * sp0m
    f1 = e2 + 4.0 * sp1m
    bs = rt2s - 2.0 * wws
    evals_mse = (A + bs) / (4.0 * Bn)
    mse_loss = (0.5 * A + 0.5 * C + D) / (3.0 * Bn)
    loss = (
        w[0] * evals_mse
        + w[1] * e1 / Bn
        + w[2] * e2 / Bn
        + w[3] * f1 / Bn
        + w[4] * f0 / Bn
        + w[5] * mse_loss
    )
    return np.float32(loss)


def kernel(y_pred: np.ndarray, y_true: np.ndarray, weights: np.ndarray) -> np.ndarray:
    return _host_combine(_get_built(), y_pred, y_true, weights)


# revision 39
# speedup vs baseline: 1.0322x; 1.0322x over previous
"""Trainium2 Bass kernel for the EighMSE loss (data-parallel over 8 cores).

Math (row (a, b, c) encodes [[a, b], [b, c]]):
  SM = a + c, DF = a - c, RT = sqrt(DF^2 + 4 b^2)
  closed-form evals = (SM +- RT) / 2,  x = clamp(DF / RT, -1, 1)
  n0 = sqrt((1 + x)/2), n1u = sqrt((1 - x)/2)
  LAPACK larger-eval eigenvector g = tau0 * (n0, s_b * n1u) with
    tau0 = -1 if DF > 0 else s_b * s_SM,  s_* = sign
  smaller-eval eigenvector = s_SM * (-g1, g0)

Sign trick: tau0 = -sign(Z) with Z = relu(DF)*2^40 - b*SM, so the pair
products need only sign(Zp*Zt) (and * sign(bp*bt) for the g1 column) —
no per-half mask chains.  Sum identities (per-core partials, f32 accums):
  RT2S = sum RT^2 (both halves)     WWS = sum sqrt(RT2p*RT2t)
  U = sum (1+xp)(1+xt)              V = sum (1-xp)(1-xt)
  SP0 = sum sgn(ZZ) * NN0           SP1 = sum sgn(ZZ*BB) * NN1
  SP0m = sum [SMp*SMt<0] * P0       SP1m = sum [SMp*SMt<0] * P1
  A = sum dSM^2   C = sum dDF^2   D = sum db^2
with NN0 = n0p*n0t = sqrt(U-term)/2, NN1 = sqrt(V-term)/2.
Host combine:
  Sx = (U - V)/2;  E1 = B + Sx/2 - 2 SP0;  E2 = B - Sx/2 - 2 SP1
  F0 = E1 + 4 SP0m;  F1 = E2 + 4 SP1m;  Bs = RT2S - 2 WWS
  loss = w0 (A+Bs)/(4B) + w1 E1/B + w2 E2/B + w3 F1/B + w4 F0/B
         + w5 (A/2 + C/2 + D)/(3B)

Engine split: Pool does SM/DF/b*SM, Act does the bf16 convert + sqrts,
DVE runs fused custom ops (square-diff-accumulate, signed-mul-accumulate,
masked-accumulate, clamped divide via 1-Newton reciprocal).
"""

import numpy as np
from contextlib import ExitStack
from operator import add as _opadd

import concourse.bass as bass
import concourse.bacc as bacc
import concourse.tile as tile
from concourse import mybir
from concourse import dve_ops as _D
from concourse.bass_utils import run_bass_kernel_spmd
from concourse.dve_spec import (
    AluOp,
    Bin,
    C0,
    C1,
    C2,
    One,
    Spec,
    Src0,
    Src1,
    Zero,
    _has_src1,
    lower,
    maxx,
    minn,
    select,
    sq,
)
from concourse.dve_uop import DveOpSpec

F32 = mybir.dt.float32
BF16 = mybir.dt.bfloat16
OP = mybir.AluOpType
AF = mybir.ActivationFunctionType

B_TOTAL = 4_194_304
NCORES = 8
S = B_TOTAL // NCORES          # samples per core
P = 128                        # partitions
NPC = S // P                   # samples per partition (4096)
W = 1024                       # sample-pairs per tile per partition
CW = 2 * W                     # combined (pred|true) tile width
NT = NPC // W                  # tiles per core
NSTAT = 7                      # chained f32 accumulator columns
BIGS = float(2.0 ** 40)        # relu(DF) scale for the sign trick

# ---- custom DVE op registration (process-local registries) -----------------

_VER = "v3"


def _acc_ref(body_fn):
    def _r(in0, in1, c0, c1, c2):
        b = body_fn(in0, in1, c0, c1, c2).astype(np.float32)
        return b, c0 + b.reshape(b.shape[0], -1).sum(axis=-1, keepdims=True)

    return _r


def _register(name, spec):
    if name in _D._SUB_OPCODE_FOR_NAME:
        return next(op for op in _D.OPS if op.name == name)
    row = _D._CUSTOM_DVE_ROW_BASE + len(_D.OPS)
    assert row <= 0x1F, f"custom-DVE row overflow: {row}"
    _D._SUB_OPCODE_FOR_NAME[name] = row
    uops = lower(spec, ver=_VER)
    sha = DveOpSpec(name=name, opcode=row, uops=uops, rd1_en=_has_src1(spec)).sha(_VER)
    op = _D.DveOp(name, spec, subdim=False, uops_sha={_VER: sha})
    _D.OPS.append(op)
    _D.CUSTOM_DVE_SPECS[name] = spec
    return op


_nx = Bin(AluOp.BITWISE_NOT, Src1, Src1)
_y0 = _nx * C0
_y1 = _y0 * (C1 - Src1 * _y0)


def _ref_xop(in0, in1, c0, c1, c2):
    nx = (~np.ascontiguousarray(in1, np.float32).view(np.int32)).view(np.float32)
    y0 = nx * np.float32(c0)
    y1 = y0 * (np.float32(c1) - in1 * y0)
    x = in0.astype(np.float32) * y1
    return np.minimum(np.maximum(x, c2), 1.0)


# out = clamp(in0 * recip1nr(in1), [imm2, 1]); in1 must be f32 (bit trick)
X_CLAMP_DIV = _register(
    "X_CLAMP_DIV", Spec(body=minn(maxx(Src0 * _y1, C2), One), reference=_ref_xop)
)
X_CONSTS = dict(s0=-0.23549792, s1=2.0017324, imm2=-1.0)

# out = in0^2 + in1^2 * s1; accum_out = s0 + sum(out)
RT2_ACC = _register(
    "RT2_ACC",
    Spec(
        body=sq(Src0) + sq(Src1) * C1,
        accum=_opadd,
        accum_init=C0,
        reference=_acc_ref(
            lambda in0, in1, c0, c1, c2: in0.astype(np.float32) ** 2
            + in1.astype(np.float32) ** 2 * c1
        ),
    ),
)

# out = (1+in0)(1+in1); accum_out = s0 + sum(out)
PAIR_U = _register(
    "PAIR_U",
    Spec(
        body=(Src0 + One) * (Src1 + One),
        accum=_opadd,
        accum_init=C0,
        reference=_acc_ref(
            lambda in0, in1, c0, c1, c2: (in0.astype(np.float32) + 1.0) * (in1 + 1.0)
        ),
    ),
)

# out = (1-in0)(1-in1); accum_out = s0 + sum(out)
PAIR_V = _register(
    "PAIR_V",
    Spec(
        body=(One - Src0) * (One - Src1),
        accum=_opadd,
        accum_init=C0,
        reference=_acc_ref(
            lambda in0, in1, c0, c1, c2: (1.0 - in0.astype(np.float32)) * (1.0 - in1)
        ),
    ),
)

# out = in0<0 ? -in1 : in1; accum_out = s0 + sum(out)
SGN_MUL_ACC = _register(
    "SGN_MUL_ACC",
    Spec(
        body=select(Src0 < Zero, Zero - Src1, Src1),
        accum=_opadd,
        accum_init=C0,
        reference=_acc_ref(
            lambda in0, in1, c0, c1, c2: np.where(
                in0 < 0, -in1.astype(np.float32), in1.astype(np.float32)
            )
        ),
    ),
)

# out = in0<0 ? in1 : 0; accum_out = s0 + sum(out)
MASK_ACC = _register(
    "MASK_ACC",
    Spec(
        body=select(Src0 < Zero, Src1, Zero),
        accum=_opadd,
        accum_init=C0,
        reference=_acc_ref(
            lambda in0, in1, c0, c1, c2: np.where(in0 < 0, in1.astype(np.float32), 0.0)
        ),
    ),
)

# out = (in0-in1)^2; accum_out = s0 + sum(out)
SQDIFF_ACC = _register(
    "SQDIFF_ACC",
    Spec(
        body=sq(Src0 - Src1),
        accum=_opadd,
        accum_init=C0,
        reference=_acc_ref(
            lambda in0, in1, c0, c1, c2: (in0.astype(np.float32) - in1) ** 2
        ),
    ),
)

# stats column indices
RT2S, UCOL, VCOL, SP0, SP1, SP0M, SP1M = range(NSTAT)

_BUILT = None


def _build_bass():
    nc = bacc.Bacc()
    yp = nc.declare_dram_parameter("y_pred", [S, 3], F32, isOutput=False)
    yt = nc.declare_dram_parameter("y_true", [S, 3], F32, isOutput=False)
    out = nc.declare_dram_parameter("out", [P, NSTAT + 4], F32, isOutput=True)

    ypr = yp.rearrange("(p n) c -> p n c", p=P)
    ytr = yt.rearrange("(p n) c -> p n c", p=P)

    with tile.TileContext(nc) as tc, ExitStack() as ctx:
        inp = ctx.enter_context(tc.tile_pool(name="inp", bufs=2))
        wk = ctx.enter_context(tc.tile_pool(name="wk", bufs=2))
        accp = ctx.enter_context(tc.tile_pool(name="accp", bufs=1))

        stats = accp.tile([P, NSTAT * (NT + 1)], F32)
        nc.vector.memset(stats[:], 0.0)
        wwacc = accp.tile([P, NT], F32)
        dacc = accp.tile([P, 3 * NT], F32)
        epsc = accp.tile([P, 1], F32)
        nc.vector.memset(epsc[:], 1e-12)

        def st(col, i):
            k = col * (NT + 1) + i
            return stats[:, k : k + 1]

        for i in range(NT):
            xall = inp.tile([P, CW, 3], F32, tag="xall")
            nc.sync.dma_start(xall[:, 0:W, :], ypr[:, bass.ts(i, W), :])
            nc.sync.dma_start(xall[:, W:CW, :], ytr[:, bass.ts(i, W), :])
            a = xall[:, :, 0]
            b = xall[:, :, 1]
            c = xall[:, :, 2]

            SM = wk.tile([P, CW], BF16, tag="SM")
            DF = wk.tile([P, CW], BF16, tag="DF")
            bb = wk.tile([P, CW], BF16, tag="bb")
            RT2 = wk.tile([P, CW], BF16, tag="RT2")
            RT = wk.tile([P, CW], F32, tag="RT")
            BS = wk.tile([P, CW], BF16, tag="BS")
            Z = wk.tile([P, CW], BF16, tag="Z")
            XC = wk.tile([P, CW], BF16, tag="XC")

            def emit_chain(sl, colidx, dve_sub):
                aa, bsl, cc = xall[:, sl, 0], xall[:, sl, 1], xall[:, sl, 2]
                nc.gpsimd.tensor_add(SM[:, sl], aa, cc)
                if dve_sub:
                    nc.vector.tensor_sub(DF[:, sl], aa, cc)
                else:
                    nc.gpsimd.tensor_sub(DF[:, sl], aa, cc)
                nc.scalar.activation(bb[:, sl], bsl, AF.Copy)
                nc.vector._custom_dve(
                    RT2_ACC, out=RT2[:, sl], in0=DF[:, sl], in1=bb[:, sl],
                    s0=0.0, s1=4.0, accum_out=st(RT2S, colidx),
                )
                nc.scalar.activation(RT[:, sl], RT2[:, sl], AF.Sqrt, bias=epsc[:])
                nc.vector.tensor_mul(BS[:, sl], bb[:, sl], SM[:, sl])
                nc.vector.tensor_scalar(
                    Z[:, sl], DF[:, sl], 0.0, BIGS, op0=OP.max, op1=OP.mult
                )
                nc.vector.tensor_sub(Z[:, sl], Z[:, sl], BS[:, sl])
                nc.vector._custom_dve(
                    X_CLAMP_DIV, out=XC[:, sl], in0=DF[:, sl], in1=RT[:, sl],
                    **X_CONSTS
                )

            if i == 0:
                emit_chain(slice(0, W), 0, True)
                emit_chain(slice(W, CW), NT, True)
            else:
                emit_chain(slice(0, CW), i, False)

            # tail: pred half [:, :W], true half [:, W:]
            def ph(t):
                return t[:, 0:W]

            def th(t):
                return t[:, W:CW]

            SS = wk.tile([P, W], BF16, tag="SS")
            nc.gpsimd.tensor_mul(SS[:], ph(SM), th(SM))
            BBp = wk.tile([P, W], BF16, tag="BBp")
            nc.gpsimd.tensor_mul(BBp[:], ph(bb), th(bb))
            da = wk.tile([P, W], BF16, tag="da")
            (nc.gpsimd if i >= 3 else nc.vector).tensor_sub(da[:], ph(SM), th(SM))
            sa = wk.tile([P, W], BF16, tag="sa")
            nc.scalar.activation(sa[:], da[:], AF.Square, accum_out=dacc[:, 3 * i : 3 * i + 1])
            dc = wk.tile([P, W], BF16, tag="dc")
            (nc.gpsimd if i >= 3 else nc.vector).tensor_sub(dc[:], ph(DF), th(DF))
            sc = wk.tile([P, W], BF16, tag="sc")
            nc.scalar.activation(sc[:], dc[:], AF.Square, accum_out=dacc[:, 3 * i + 1 : 3 * i + 2])
            dd = wk.tile([P, W], BF16, tag="dd")
            (nc.gpsimd if i >= 3 else nc.vector).tensor_sub(dd[:], ph(bb), th(bb))
            sd = wk.tile([P, W], BF16, tag="sd")
            nc.scalar.activation(sd[:], dd[:], AF.Square, accum_out=dacc[:, 3 * i + 2 : 3 * i + 3])
            ww = wk.tile([P, W], BF16, tag="ww")
            nc.gpsimd.tensor_mul(ww[:], ph(RT2), th(RT2))
            wws = wk.tile([P, W], BF16, tag="wws")
            nc.scalar.activation(
                wws[:], ww[:], AF.Sqrt, accum_out=wwacc[:, i : i + 1]
            )
            u = wk.tile([P, W], BF16, tag="u")
            nc.vector._custom_dve(
                PAIR_U, out=u[:], in0=ph(XC), in1=th(XC),
                s0=0.0, accum_out=st(UCOL, i),
            )
            v = wk.tile([P, W], BF16, tag="v")
            nc.vector._custom_dve(
                PAIR_V, out=v[:], in0=ph(XC), in1=th(XC),
                s0=0.0, accum_out=st(VCOL, i),
            )
            NN0 = wk.tile([P, W], BF16, tag="NN0")
            nc.scalar.activation(NN0[:], u[:], AF.Sqrt, scale=0.25)
            NN1 = wk.tile([P, W], BF16, tag="NN1")
            nc.scalar.activation(NN1[:], v[:], AF.Sqrt, scale=0.25)

            ZZ = wk.tile([P, W], BF16, tag="ZZ")
            (nc.gpsimd if i >= 3 else nc.vector).tensor_mul(ZZ[:], ph(Z), th(Z))
            P0 = wk.tile([P, W], BF16, tag="P0")
            nc.vector._custom_dve(
                SGN_MUL_ACC, out=P0[:], in0=ZZ[:], in1=NN0[:],
                s0=0.0, accum_out=st(SP0, i),
            )
            scr = wk.tile([P, W], BF16, tag="scr")
            nc.vector._custom_dve(
                MASK_ACC, out=scr[:], in0=SS[:], in1=P0[:],
                s0=0.0, accum_out=st(SP0M, i),
            )
            nc.vector.tensor_mul(BBp[:], ZZ[:], BBp[:])  # ZB in place
            P1 = wk.tile([P, W], BF16, tag="P1")
            nc.vector._custom_dve(
                SGN_MUL_ACC, out=P1[:], in0=BBp[:], in1=NN1[:],
                s0=0.0, accum_out=st(SP1, i),
            )
            scr2 = wk.tile([P, W], BF16, tag="scr2")
            nc.vector._custom_dve(
                MASK_ACC, out=scr2[:], in0=SS[:], in1=P1[:],
                s0=0.0, accum_out=st(SP1M, i),
            )



        outsums = accp.tile([P, NSTAT + 4], F32)
        stats3 = stats[:].rearrange("p (c t) -> p c t", c=NSTAT)
        rscr = accp.tile([P, NT + 1], F32)
        for cidx in range(NSTAT):
            nc.scalar.activation(
                rscr[:], stats3[:, cidx, :], AF.Copy,
                accum_out=outsums[:, cidx : cidx + 1],
            )
        rscr2 = accp.tile([P, NT], F32)
        nc.scalar.activation(
            rscr2[:], wwacc[:], AF.Copy, accum_out=outsums[:, NSTAT : NSTAT + 1]
        )
        dacc3 = dacc[:].rearrange("p (t k) -> p k t", k=3)
        rscr3 = accp.tile([P, NT], F32)
        for k in range(3):
            nc.scalar.activation(
                rscr3[:], dacc3[:, k, :], AF.Copy,
                accum_out=outsums[:, NSTAT + 1 + k : NSTAT + 2 + k],
            )
        nc.sync.dma_start(out[:, :], outsums[:])

    nc.compile()
    return nc


def _get_built():
    global _BUILT
    if _BUILT is None:
        _BUILT = _build_bass()
    return _BUILT


def _host_combine(nc, y_pred, y_true, weights):
    y_pred = np.ascontiguousarray(y_pred, dtype=np.float32)
    y_true = np.ascontiguousarray(y_true, dtype=np.float32)
    w = np.asarray(weights, dtype=np.float64)

    in_maps = []
    for cid in range(NCORES):
        in_maps.append(
            {
                "y_pred": y_pred[cid * S : (cid + 1) * S],
                "y_true": y_true[cid * S : (cid + 1) * S],
            }
        )
    res = run_bass_kernel_spmd(nc, in_maps, list(range(NCORES)))
    sums = np.zeros(NSTAT + 4, dtype=np.float64)
    for cid in range(NCORES):
        sums += np.asarray(res.results[cid]["out"], dtype=np.float64).sum(axis=0)

    rt2s, u, v, sp0, sp1, sp0m, sp1m, wws, A, C, D = sums
    Bn = float(B_TOTAL)
    sx = (u - v) / 2.0
    e1 = Bn + sx / 2.0 - 2.0 * sp0
    e2 = Bn - sx / 2.0 - 2.0 * sp1
    f0 = e1 + 4.0 * sp0m
    f1 = e2 + 4.0 * sp1m
    bs = rt2s - 2.0 * wws
    evals_mse = (A + bs) / (4.0 * Bn)
    mse_loss = (0.5 * A + 0.5 * C + D) / (3.0 * Bn)
    loss = (
        w[0] * evals_mse
        + w[1] * e1 / Bn
        + w[2] * e2 / Bn
        + w[3] * f1 / Bn
        + w[4] * f0 / Bn
        + w[5] * mse_loss
    )
    return np.float32(loss)


def kernel(y_pred: np.ndarray, y_true: np.ndarray, weights: np.ndarray) -> np.ndarray:
    return _host_combine(_get_built(), y_pred, y_true, weights)


# revision 47
# speedup vs baseline: 1.0498x; 1.0171x over previous
"""Trainium2 Bass kernel for the EighMSE loss (data-parallel over 8 cores).

Math (row (a, b, c) encodes [[a, b], [b, c]]):
  SM = a + c, DF = a - c, RT = sqrt(DF^2 + 4 b^2)
  closed-form evals = (SM +- RT) / 2,  x = clamp(DF / RT, -1, 1)
  n0 = sqrt((1 + x)/2), n1u = sqrt((1 - x)/2)
  LAPACK larger-eval eigenvector g = tau0 * (n0, s_b * n1u) with
    tau0 = -1 if DF > 0 else s_b * s_SM,  s_* = sign
  smaller-eval eigenvector = s_SM * (-g1, g0)

Sign trick: tau0 = -sign(Z) with Z = relu(DF)*2^40 - b*SM, so the pair
products need only sign(Zp*Zt) (and * sign(bp*bt) for the g1 column) —
no per-half mask chains.  Sum identities (per-core partials, f32 accums):
  RT2S = sum RT^2 (both halves)     WWS = sum sqrt(RT2p*RT2t)
  U = sum (1+xp)(1+xt)              V = sum (1-xp)(1-xt)
  SP0 = sum sgn(ZZ) * NN0           SP1 = sum sgn(ZZ*BB) * NN1
  SP0m = sum [SMp*SMt<0] * P0       SP1m = sum [SMp*SMt<0] * P1
  A = sum dSM^2   C = sum dDF^2   D = sum db^2
with NN0 = n0p*n0t = sqrt(U-term)/2, NN1 = sqrt(V-term)/2.
Host combine:
  Sx = (U - V)/2;  E1 = B + Sx/2 - 2 SP0;  E2 = B - Sx/2 - 2 SP1
  F0 = E1 + 4 SP0m;  F1 = E2 + 4 SP1m;  Bs = RT2S - 2 WWS
  loss = w0 (A+Bs)/(4B) + w1 E1/B + w2 E2/B + w3 F1/B + w4 F0/B
         + w5 (A/2 + C/2 + D)/(3B)

Engine split: Pool does SM/DF/b*SM, Act does the bf16 convert + sqrts,
DVE runs fused custom ops (square-diff-accumulate, signed-mul-accumulate,
masked-accumulate, clamped divide via 1-Newton reciprocal).
"""

import numpy as np
from contextlib import ExitStack
from operator import add as _opadd

import concourse.bass as bass
import concourse.bacc as bacc
import concourse.tile as tile
from concourse import mybir
from concourse import dve_ops as _D
from concourse.bass_utils import run_bass_kernel_spmd
from concourse.dve_spec import (
    AluOp,
    Bin,
    C0,
    C1,
    C2,
    One,
    Spec,
    Src0,
    Src1,
    Zero,
    _has_src1,
    lower,
    maxx,
    minn,
    select,
    sq,
)
from concourse.dve_uop import DveOpSpec

F32 = mybir.dt.float32
BF16 = mybir.dt.bfloat16
OP = mybir.AluOpType
AF = mybir.ActivationFunctionType

B_TOTAL = 4_194_304
NCORES = 8
S = B_TOTAL // NCORES          # samples per core
P = 128                        # partitions
NPC = S // P                   # samples per partition (4096)
W = 1024                       # sample-pairs per tile per partition
CW = 2 * W                     # combined (pred|true) tile width
NT = NPC // W                  # tiles per core
NSTAT = 7                      # chained f32 accumulator columns
BIGS = float(2.0 ** 40)        # relu(DF) scale for the sign trick

# ---- custom DVE op registration (process-local registries) -----------------

_VER = "v3"


def _acc_ref(body_fn):
    def _r(in0, in1, c0, c1, c2):
        b = body_fn(in0, in1, c0, c1, c2).astype(np.float32)
        return b, c0 + b.reshape(b.shape[0], -1).sum(axis=-1, keepdims=True)

    return _r


def _register(name, spec):
    if name in _D._SUB_OPCODE_FOR_NAME:
        return next(op for op in _D.OPS if op.name == name)
    row = _D._CUSTOM_DVE_ROW_BASE + len(_D.OPS)
    assert row <= 0x1F, f"custom-DVE row overflow: {row}"
    _D._SUB_OPCODE_FOR_NAME[name] = row
    uops = lower(spec, ver=_VER)
    sha = DveOpSpec(name=name, opcode=row, uops=uops, rd1_en=_has_src1(spec)).sha(_VER)
    op = _D.DveOp(name, spec, subdim=False, uops_sha={_VER: sha})
    _D.OPS.append(op)
    _D.CUSTOM_DVE_SPECS[name] = spec
    return op


_nx = Bin(AluOp.BITWISE_NOT, Src1, Src1)
_y0 = _nx * C0
_y1 = _y0 * (C1 - Src1 * _y0)


def _ref_xop(in0, in1, c0, c1, c2):
    nx = (~np.ascontiguousarray(in1, np.float32).view(np.int32)).view(np.float32)
    y0 = nx * np.float32(c0)
    y1 = y0 * (np.float32(c1) - in1 * y0)
    x = in0.astype(np.float32) * y1
    return np.minimum(np.maximum(x, c2), 1.0)


# out = clamp(in0 * recip1nr(in1), [imm2, 1]); in1 must be f32 (bit trick)
X_CLAMP_DIV = _register(
    "X_CLAMP_DIV", Spec(body=minn(maxx(Src0 * _y1, C2), One), reference=_ref_xop)
)
X_CONSTS = dict(s0=-0.23549792, s1=2.0017324, imm2=-1.0)

# out = in0^2 + in1^2 * s1; accum_out = s0 + sum(out)
RT2_ACC = _register(
    "RT2_ACC",
    Spec(
        body=sq(Src0) + sq(Src1) * C1,
        accum=_opadd,
        accum_init=C0,
        reference=_acc_ref(
            lambda in0, in1, c0, c1, c2: in0.astype(np.float32) ** 2
            + in1.astype(np.float32) ** 2 * c1
        ),
    ),
)

# out = (1+in0)(1+in1); accum_out = s0 + sum(out)
PAIR_U = _register(
    "PAIR_U",
    Spec(
        body=(Src0 + One) * (Src1 + One),
        accum=_opadd,
        accum_init=C0,
        reference=_acc_ref(
            lambda in0, in1, c0, c1, c2: (in0.astype(np.float32) + 1.0) * (in1 + 1.0)
        ),
    ),
)

# out = (1-in0)(1-in1); accum_out = s0 + sum(out)
PAIR_V = _register(
    "PAIR_V",
    Spec(
        body=(One - Src0) * (One - Src1),
        accum=_opadd,
        accum_init=C0,
        reference=_acc_ref(
            lambda in0, in1, c0, c1, c2: (1.0 - in0.astype(np.float32)) * (1.0 - in1)
        ),
    ),
)

# out = in0<0 ? -in1 : in1; accum_out = s0 + sum(out)
SGN_MUL_ACC = _register(
    "SGN_MUL_ACC",
    Spec(
        body=select(Src0 < Zero, Zero - Src1, Src1),
        accum=_opadd,
        accum_init=C0,
        reference=_acc_ref(
            lambda in0, in1, c0, c1, c2: np.where(
                in0 < 0, -in1.astype(np.float32), in1.astype(np.float32)
            )
        ),
    ),
)

# out = in0<0 ? in1 : 0; accum_out = s0 + sum(out)
MASK_ACC = _register(
    "MASK_ACC",
    Spec(
        body=select(Src0 < Zero, Src1, Zero),
        accum=_opadd,
        accum_init=C0,
        reference=_acc_ref(
            lambda in0, in1, c0, c1, c2: np.where(in0 < 0, in1.astype(np.float32), 0.0)
        ),
    ),
)

# out = (in0-in1)^2; accum_out = s0 + sum(out)
SQDIFF_ACC = _register(
    "SQDIFF_ACC",
    Spec(
        body=sq(Src0 - Src1),
        accum=_opadd,
        accum_init=C0,
        reference=_acc_ref(
            lambda in0, in1, c0, c1, c2: (in0.astype(np.float32) - in1) ** 2
        ),
    ),
)

# stats column indices
RT2S, UCOL, VCOL, SP0, SP1, SP0M, SP1M = range(NSTAT)

_BUILT = None


def _build_bass():
    nc = bacc.Bacc()
    yp = nc.declare_dram_parameter("y_pred", [S, 3], F32, isOutput=False)
    yt = nc.declare_dram_parameter("y_true", [S, 3], F32, isOutput=False)
    out = nc.declare_dram_parameter("out", [P, NSTAT + 4], F32, isOutput=True)

    ypr = yp.rearrange("(p n) c -> p n c", p=P)
    ytr = yt.rearrange("(p n) c -> p n c", p=P)

    with tile.TileContext(nc) as tc, ExitStack() as ctx:
        inp = ctx.enter_context(tc.tile_pool(name="inp", bufs=2))
        wk = ctx.enter_context(tc.tile_pool(name="wk", bufs=2))
        accp = ctx.enter_context(tc.tile_pool(name="accp", bufs=1))

        stats = accp.tile([P, NSTAT * (NT + 1)], F32)
        nc.vector.memset(stats[:], 0.0)
        wwacc = accp.tile([P, NT], F32)
        dacc = accp.tile([P, 3 * NT], F32)
        epsc = accp.tile([P, 1], F32)
        nc.vector.memset(epsc[:], 1e-12)

        def st(col, i):
            k = col * (NT + 1) + i
            return stats[:, k : k + 1]

        for i in range(NT):
            xall = inp.tile([P, CW, 3], F32, tag="xall")
            nc.sync.dma_start(xall[:, 0:W, :], ypr[:, bass.ts(i, W), :])
            nc.sync.dma_start(xall[:, W:CW, :], ytr[:, bass.ts(i, W), :])
            a = xall[:, :, 0]
            b = xall[:, :, 1]
            c = xall[:, :, 2]

            SM = wk.tile([P, CW], BF16, tag="SM")
            DF = wk.tile([P, CW], BF16, tag="DF")
            bb = wk.tile([P, CW], BF16, tag="bb")
            RT2 = wk.tile([P, CW], BF16, tag="RT2")
            RT = wk.tile([P, CW], F32, tag="RT")
            BS = wk.tile([P, CW], BF16, tag="BS")
            Z = wk.tile([P, CW], BF16, tag="Z")
            XC = wk.tile([P, CW], BF16, tag="XC")

            def emit_chain(sl, colidx, dve_sub):
                aa, bsl, cc = xall[:, sl, 0], xall[:, sl, 1], xall[:, sl, 2]
                nc.gpsimd.tensor_add(SM[:, sl], aa, cc)
                if dve_sub or colidx == 2:
                    nc.vector.tensor_sub(DF[:, sl], aa, cc)
                else:
                    nc.gpsimd.tensor_sub(DF[:, sl], aa, cc)
                nc.scalar.activation(bb[:, sl], bsl, AF.Copy)
                nc.vector._custom_dve(
                    RT2_ACC, out=RT2[:, sl], in0=DF[:, sl], in1=bb[:, sl],
                    s0=0.0, s1=4.0, accum_out=st(RT2S, colidx),
                )
                nc.scalar.activation(RT[:, sl], RT2[:, sl], AF.Sqrt, bias=epsc[:])
                nc.vector.tensor_mul(BS[:, sl], bb[:, sl], SM[:, sl])
                nc.vector.tensor_scalar(
                    Z[:, sl], DF[:, sl], 0.0, BIGS, op0=OP.max, op1=OP.mult
                )
                nc.vector.tensor_sub(Z[:, sl], Z[:, sl], BS[:, sl])
                nc.vector._custom_dve(
                    X_CLAMP_DIV, out=XC[:, sl], in0=DF[:, sl], in1=RT[:, sl],
                    **X_CONSTS
                )

            if i == 0:
                emit_chain(slice(0, W), 0, True)
                emit_chain(slice(W, CW), NT, True)
            else:
                emit_chain(slice(0, CW), i, False)

            # tail: pred half [:, :W], true half [:, W:]
            def ph(t):
                return t[:, 0:W]

            def th(t):
                return t[:, W:CW]

            SS = wk.tile([P, W], BF16, tag="SS")
            nc.gpsimd.tensor_mul(SS[:], ph(SM), th(SM))
            BBp = wk.tile([P, W], BF16, tag="BBp")
            nc.gpsimd.tensor_mul(BBp[:], ph(bb), th(bb))
            da = wk.tile([P, W], BF16, tag="da")
            (nc.gpsimd if i >= 3 else nc.vector).tensor_sub(da[:], ph(SM), th(SM))
            sa = wk.tile([P, W], BF16, tag="sa")
            nc.scalar.activation(sa[:], da[:], AF.Square, accum_out=dacc[:, 3 * i : 3 * i + 1])
            dc = wk.tile([P, W], BF16, tag="dc")
            (nc.gpsimd if i >= 3 else nc.vector).tensor_sub(dc[:], ph(DF), th(DF))
            sc = wk.tile([P, W], BF16, tag="sc")
            nc.scalar.activation(sc[:], dc[:], AF.Square, accum_out=dacc[:, 3 * i + 1 : 3 * i + 2])
            dd = wk.tile([P, W], BF16, tag="dd")
            (nc.gpsimd if i >= 3 else nc.vector).tensor_sub(dd[:], ph(bb), th(bb))
            sd = wk.tile([P, W], BF16, tag="sd")
            nc.scalar.activation(sd[:], dd[:], AF.Square, accum_out=dacc[:, 3 * i + 2 : 3 * i + 3])
            ww = wk.tile([P, W], BF16, tag="ww")
            nc.gpsimd.tensor_mul(ww[:], ph(RT2), th(RT2))
            wws = wk.tile([P, W], BF16, tag="wws")
            nc.scalar.activation(
                wws[:], ww[:], AF.Sqrt, accum_out=wwacc[:, i : i + 1]
            )
            u = wk.tile([P, W], BF16, tag="u")
            nc.vector._custom_dve(
                PAIR_U, out=u[:], in0=ph(XC), in1=th(XC),
                s0=0.0, accum_out=st(UCOL, i),
            )
            v = wk.tile([P, W], BF16, tag="v")
            nc.vector._custom_dve(
                PAIR_V, out=v[:], in0=ph(XC), in1=th(XC),
                s0=0.0, accum_out=st(VCOL, i),
            )
            NN0 = wk.tile([P, W], BF16, tag="NN0")
            nc.scalar.activation(NN0[:], u[:], AF.Sqrt, scale=0.25)
            NN1 = wk.tile([P, W], BF16, tag="NN1")
            nc.scalar.activation(NN1[:], v[:], AF.Sqrt, scale=0.25)

            ZZ = wk.tile([P, W], BF16, tag="ZZ")
            (nc.gpsimd if i >= 3 else nc.vector).tensor_mul(ZZ[:], ph(Z), th(Z))
            P0 = wk.tile([P, W], BF16, tag="P0")
            nc.vector._custom_dve(
                SGN_MUL_ACC, out=P0[:], in0=ZZ[:], in1=NN0[:],
                s0=0.0, accum_out=st(SP0, i),
            )
            scr = wk.tile([P, W], BF16, tag="scr")
            nc.vector._custom_dve(
                MASK_ACC, out=scr[:], in0=SS[:], in1=P0[:],
                s0=0.0, accum_out=st(SP0M, i),
            )
            nc.vector.tensor_mul(BBp[:], ZZ[:], BBp[:])  # ZB in place
            P1 = wk.tile([P, W], BF16, tag="P1")
            nc.vector._custom_dve(
                SGN_MUL_ACC, out=P1[:], in0=BBp[:], in1=NN1[:],
                s0=0.0, accum_out=st(SP1, i),
            )
            scr2 = wk.tile([P, W], BF16, tag="scr2")
            nc.vector._custom_dve(
                MASK_ACC, out=scr2[:], in0=SS[:], in1=P1[:],
                s0=0.0, accum_out=st(SP1M, i),
            )



        outsums = accp.tile([P, NSTAT + 4], F32)
        stats3 = stats[:].rearrange("p (c t) -> p c t", c=NSTAT)
        rscr = accp.tile([P, NT + 1], F32)
        for cidx in range(NSTAT):
            nc.scalar.activation(
                rscr[:], stats3[:, cidx, :], AF.Copy,
                accum_out=outsums[:, cidx : cidx + 1],
            )
        rscr2 = accp.tile([P, NT], F32)
        nc.scalar.activation(
            rscr2[:], wwacc[:], AF.Copy, accum_out=outsums[:, NSTAT : NSTAT + 1]
        )
        dacc3 = dacc[:].rearrange("p (t k) -> p k t", k=3)
        rscr3 = accp.tile([P, NT], F32)
        for k in range(3):
            nc.scalar.activation(
                rscr3[:], dacc3[:, k, :], AF.Copy,
                accum_out=outsums[:, NSTAT + 1 + k : NSTAT + 2 + k],
            )
        nc.sync.dma_start(out[:, :], outsums[:])

    nc.compile()
    return nc


def _get_built():
    global _BUILT
    if _BUILT is None:
        _BUILT = _build_bass()
    return _BUILT


def _host_combine(nc, y_pred, y_true, weights):
    y_pred = np.ascontiguousarray(y_pred, dtype=np.float32)
    y_true = np.ascontiguousarray(y_true, dtype=np.float32)
    w = np.asarray(weights, dtype=np.float64)

    in_maps = []
    for cid in range(NCORES):
        in_maps.append(
            {
                "y_pred": y_pred[cid * S : (cid + 1) * S],
                "y_true": y_true[cid * S : (cid + 1) * S],
            }
        )
    res = run_bass_kernel_spmd(nc, in_maps, list(range(NCORES)))
    sums = np.zeros(NSTAT + 4, dtype=np.float64)
    for cid in range(NCORES):
        sums += np.asarray(res.results[cid]["out"], dtype=np.float64).sum(axis=0)

    rt2s, u, v, sp0, sp1, sp0m, sp1m, wws, A, C, D = sums
    Bn = float(B_TOTAL)
    sx = (u - v) / 2.0
    e1 = Bn + sx / 2.0 - 2.0 * sp0
    e2 = Bn - sx / 2.0 - 2.0 * sp1
    f0 = e1 + 4.0 * sp0m
    f1 = e2 + 4.0 * sp1m
    bs = rt2s - 2.0 * wws
    evals_mse = (A + bs) / (4.0 * Bn)
    mse_loss = (0.5 * A + 0.5 * C + D) / (3.0 * Bn)
    loss = (
        w[0] * evals_mse
        + w[1] * e1 / Bn
        + w[2] * e2 / Bn
        + w[3] * f1 / Bn
        + w[4] * f0 / Bn
        + w[5] * mse_loss
    )
    return np.float32(loss)


def kernel(y_pred: np.ndarray, y_true: np.ndarray, weights: np.ndarray) -> np.ndarray:
    return _host_combine(_get_built(), y_pred, y_true, weights)
